# revision 1
# baseline (speedup 1.0000x reference)
"""Trainium2 Bass kernel for HadamardPackedLinear.

Math (reference):
    y[t, 128*h + o] = beta[o] * sum_g Hn[g,h] * sum_i (sum_g' x[t,128g'+i] Hn[g',g]) * w[g,o,i]
with Hn = H_pm / sqrt(32), H_pm the +-1 Sylvester Hadamard, w ternary.

We compute three structured PE stages per 128-token slab, all with the
contraction dim on SBUF partitions, using +-1 Hadamards and folding the
1/32 normalization and beta into the stage-3 moving operand:

  stage1: lhsT1[(d,g),(d',h)] = delta_dd' * Hpm[g,h]        (I4 (x) H packing)
          rhs = x packed [p=32d+g, f=t*32+i_hi]  (host pre-shuffled)
          out = psum[(d,h), (t,i_hi)] --DVE 32x32 transpose--> xm[i=32d+i_hi, t*32+h]
  stage2: per h: lhsT = W2[:, h] = w[h].T  [i, o], rhs = xm[:, h::32] (N=tokens)
          out = psum[o, t] --ACT copy--> yp_sb[o, t*32+h]
          --DVE transpose--> yp_t[32d3+h, t*32+o_hi]   (o = 32*d3 + o_hi)
  stage3: per o_hi: lhsT = yp_t[:, o_hi::32] [(d3,h), t]  (activations stationary)
          rhs = W3[:, o_hi] [(d3,h), (d3',h')] = delta * Hpm[h,h'] * beta[o]/32
          out = psum[t, (d3',h')] --ACT copy--> y_out[t, 128h'+32d3'+o_hi] --DMA-->

Sharding: data-parallel over tokens, 8 cores x 1024 tokens. No collectives.
"""

import sys

for _p in ("/opt/trn_rl_repo", "/root/.axon_site/_ro/trn_rl_repo"):
    if _p not in sys.path:
        sys.path.append(_p)

import numpy as np

import concourse.bass as bass  # noqa: E402
import concourse.mybir as mybir  # noqa: E402
import concourse.tile as tile  # noqa: E402
from concourse import bacc  # noqa: E402
from concourse.bass_utils import run_bass_kernel_spmd  # noqa: E402

F32 = mybir.dt.float32
F32R = mybir.dt.float32r
F16 = mybir.dt.float16

N_CORES = 8
B, T, D = 4, 2048, 4096
A = 32            # algebra dim (hadamard size)
IN_O = 128        # i per group
OUT_O = 128       # o per group
TOK = (B * T) // N_CORES   # tokens per core = 1024
SLAB = 128                 # tokens per slab
NSLAB = TOK // SLAB        # 8

_CACHE = {}


def _build_program():
    nc = bacc.Bacc(None, target_bir_lowering=False)

    x_d = nc.dram_tensor("x_shuf", [128, TOK * 32], F32R, kind="ExternalInput")
    h1_d = nc.dram_tensor("h1m", [128, 128], F32R, kind="ExternalInput")
    w2_d = nc.dram_tensor("w2m", [128, A * OUT_O], F16, kind="ExternalInput")
    w3_d = nc.dram_tensor("w3m", [128, 32 * 128], F16, kind="ExternalInput")
    y_d = nc.dram_tensor("y", [TOK, D], F32, kind="ExternalOutput")

    with tile.TileContext(nc) as tc:
        with (
            tc.tile_pool(name="const", bufs=1) as constp,
            tc.tile_pool(name="xin", bufs=2) as xinp,
            tc.tile_pool(name="xmst", bufs=4) as xmstp,
            tc.tile_pool(name="xm", bufs=2) as xmp,
            tc.tile_pool(name="ypsb", bufs=1) as ypsbp,
            tc.tile_pool(name="ypt", bufs=2) as yptp,
            tc.tile_pool(name="yout", bufs=2) as youtp,
            tc.tile_pool(name="ps1", bufs=2, space="PSUM") as ps1p,
            tc.tile_pool(name="ps2", bufs=2, space="PSUM") as ps2p,
            tc.tile_pool(name="ps3", bufs=2, space="PSUM") as ps3p,
        ):
            h1_t = constp.tile([128, 128], F32R)
            nc.sync.dma_start(out=h1_t[:], in_=h1_d[:])
            w2_t = constp.tile([128, A * OUT_O], F16)
            nc.sync.dma_start(out=w2_t[:], in_=w2_d[:])
            w3_t = constp.tile([128, 32 * 128], F16)
            nc.sync.dma_start(out=w3_t[:], in_=w3_d[:])

            for s in range(NSLAB):
                # ---- load x slab: [p=(d,g), f=t*32+i_hi], t in [0,128)
                x_t = xinp.tile([128, SLAB * 32], F32R)
                nc.sync.dma_start(
                    out=x_t[:], in_=x_d[:, s * SLAB * 32 : (s + 1) * SLAB * 32]
                )

                # ---- stage 1: hadamard over g (4-way delta-packed), K=128
                xm_t = xmp.tile([128, SLAB * 32], F16)
                for n in range(8):
                    ps1 = ps1p.tile([128, 512], F32)
                    nc.tensor.matmul(
                        ps1[:],
                        h1_t[:],
                        x_t[:, n * 512 : (n + 1) * 512],
                        start=True,
                        stop=True,
                    )
                    # cast fp32 psum -> fp16 staging, then 32x32 transpose:
                    # psum[(d,h),(t,i_hi)] -> xm[(d,i_hi), (t,h)]
                    xst = xmstp.tile([128, 512], F16)
                    nc.any.tensor_copy(xst[:], ps1[:])
                    nc.vector.transpose(xm_t[:, n * 512 : (n + 1) * 512], xst[:])

                # ---- stage 2: per-group ternary matmul, K=i=128
                yp_sb = ypsbp.tile([128, SLAB * 32], F16)
                xm_v = xm_t[:].rearrange("p (t h) -> p h t", h=32)
                ypsb_v = yp_sb[:].rearrange("p (t h) -> p h t", h=32)
                for hg in range(8):
                    ps2 = ps2p.tile([128, 512], F32)
                    for hl in range(4):
                        h = hg * 4 + hl
                        nc.tensor.matmul(
                            ps2[:, hl * 128 : (hl + 1) * 128],
                            w2_t[:, h * 128 : (h + 1) * 128],
                            xm_v[:, h : h + 1, :],
                            start=True,
                            stop=True,
                        )
                    # psum[o, (hl,t)] -> yp_sb[o, t*32 + (4hg+hl)] (cast fp16)
                    nc.any.tensor_copy(
                        ypsb_v[:, hg * 4 : (hg + 1) * 4, :],
                        ps2[:].rearrange("p (hl t) -> p hl t", hl=4),
                    )

                # ---- shuffle: yp_sb[o, (t,h)] -> yp_t[(d3,h), (t,o_hi)]
                yp_t = yptp.tile([128, SLAB * 32], F16)
                for n in range(8):
                    nc.vector.transpose(
                        yp_t[:, n * 512 : (n + 1) * 512],
                        yp_sb[:, n * 512 : (n + 1) * 512],
                    )

                # ---- stage 3: hadamard over h + beta, activations stationary
                y_o = youtp.tile([128, D], F32)
                ypt_v = yp_t[:].rearrange("p (t o) -> p o t", o=32)
                yo_v = y_o[:].rearrange(
                    "p (hp dp rr) -> p rr dp hp", hp=32, dp=4, rr=32
                )
                for qg in range(8):
                    ps3 = ps3p.tile([128, 512], F32)
                    for r in range(4):
                        ohi = qg * 4 + r
                        nc.tensor.matmul(
                            ps3[:, r * 128 : (r + 1) * 128],
                            ypt_v[:, ohi : ohi + 1, :],
                            w3_t[:, ohi * 128 : (ohi + 1) * 128],
                            start=True,
                            stop=True,
                        )
                    # psum[t, (r,(d3',h'))] -> y_out[t, 128h'+32d3'+(4qg+r)]
                    nc.any.tensor_copy(
                        yo_v[:, qg * 4 : (qg + 1) * 4, :, :],
                        ps3[:].rearrange("p (r dp hp) -> p r dp hp", r=4, dp=4),
                    )

                nc.sync.dma_start(
                    out=y_d[s * SLAB : (s + 1) * SLAB, :], in_=y_o[:]
                )

    nc.compile()
    return nc


def _host_prep(x, weight_packed, beta, H):
    """Build per-core shuffled x and the three structured operand matrices."""
    x = np.asarray(x, dtype=np.float32)
    weight_packed = np.asarray(weight_packed, dtype=np.uint8)
    beta = np.asarray(beta, dtype=np.float32)
    H = np.asarray(H, dtype=np.float32)

    hpm = np.where(H > 0, 1.0, -1.0).astype(np.float32)  # +-1 hadamard, symmetric

    # unpack ternary weights exactly like the reference
    p = weight_packed
    v0 = ((p >> 6) & 3).astype(np.int8) - 1
    v1 = ((p >> 4) & 3).astype(np.int8) - 1
    v2 = ((p >> 2) & 3).astype(np.int8) - 1
    v3 = (p & 3).astype(np.int8) - 1
    w = np.stack([v0, v1, v2, v3], axis=-1).reshape(A, OUT_O, IN_O).astype(np.float32)

    # stage 1 stationary: lhsT1[32d+g, 32d'+h] = delta_dd' * hpm[g,h]
    h1m = np.zeros((4, A, 4, A), dtype=np.float32)
    for d in range(4):
        h1m[d, :, d, :] = hpm
    h1m = h1m.reshape(128, 128)

    # stage 2 stationary blocks: w2m[i, 128h+o] = w[h, o, i]  (ternary, fp16 exact)
    w2m = np.ascontiguousarray(
        w.transpose(2, 0, 1).reshape(IN_O, A * OUT_O)
    ).astype(np.float16)

    # stage 3 moving blocks:
    # w3m[32*d3+h, 128*ohi + 32*d3p + hp] = delta_{d3,d3p} hpm[h,hp] beta[32*d3p+ohi]/32
    w3m = np.zeros((4, A, 32, 4, A), dtype=np.float32)
    for d3 in range(4):
        for ohi in range(32):
            w3m[d3, :, ohi, d3, :] = hpm * (beta[32 * d3 + ohi] / 32.0)
    w3m = w3m.reshape(128, 32 * 128).astype(np.float16)

    # per-core pre-shuffled x: xc[32d+g, t*32+i_hi] = x[t0+t, 128g+32d+i_hi]
    # pre-rounded to fp32r (11-bit mantissa, round-to-nearest-even)
    xf = _round_fp32r(x.reshape(B * T, D))
    x_shards = []
    for c in range(N_CORES):
        xc = xf[c * TOK : (c + 1) * TOK]  # [TOK, 4096]
        xc = xc.reshape(TOK, A, 4, 32).transpose(2, 1, 0, 3)  # [d, g, t, i_hi]
        x_shards.append(np.ascontiguousarray(xc.reshape(128, TOK * 32)))

    return x_shards, h1m, w2m, w3m


def _round_fp32r(a):
    """Round fp32 array to fp32r: mantissa 11 bits, round-to-nearest-even."""
    u = a.view(np.uint32).astype(np.uint64)
    shift = 12
    r = (u + ((1 << (shift - 1)) - 1) + ((u >> shift) & 1)) >> shift << shift
    return (r & 0xFFFFFFFF).astype(np.uint32).view(np.float32)


def kernel(x, weight_packed, beta, H):
    x_shards, h1m, w2m, w3m = _host_prep(x, weight_packed, beta, H)

    if "nc" not in _CACHE:
        _CACHE["nc"] = _build_program()
    nc = _CACHE["nc"]

    in_maps = [
        {"x_shuf": x_shards[c], "h1m": h1m, "w2m": w2m, "w3m": w3m}
        for c in range(N_CORES)
    ]
    res = run_bass_kernel_spmd(nc, in_maps, core_ids=list(range(N_CORES)))
    y = np.concatenate([res.results[c]["y"] for c in range(N_CORES)], axis=0)
    return y.reshape(B, T, D).astype(np.float32)



# revision 3
# speedup vs baseline: 1.3302x; 1.3302x over previous
"""Trainium2 Bass kernel for HadamardPackedLinear.

Math (reference):
    y[t, 128*h + o] = beta[o]/32 * sum_g Hpm[g,h] * sum_i xm[t,g,i] * w[g,o,i]
    xm[t,g,i] = sum_g' x[t, 128g'+i] * Hpm[g',g]
with Hpm the +-1 Sylvester Hadamard (H = Hpm/sqrt(32), applied twice -> 1/32).

Three PE stages per 256-token chunk (4 chunks/core), everything fp16 on the
PE with fp32 PSUM accumulate:

  S1: lhsT = I4 (x) Hpm  [ (d,g) x (d,h) ],  rhs = x [ (d,g), (u,i_hi,v) ]
      (host pre-shuffles x; tokens t = 2u+v so fp16 pairs over v are
      memory-adjacent)  ->  psum1[(d,h), (u,i_hi,v)]
  drain1 (ACT): psum1 -> xm1 fp16, contiguous multibank copies
  T1 (DVE, fp32-bitcast pair transpose): xm1 -> xm2[i, (u,h,v)]
      (32x32 block transpose in fp32 units moves 2 fp16/lane-cycle)
  S2: per g: lhsT = w2'[g] = w[g].T * beta/32  [i, o],
      rhs = xm2 cols {64u + 2g + v}  (2-elem runs, stride 64)
      -> psum2[o, t] per g, 8 g per 4-bank psum tile
  gather2 (ACT): multibank (t-outer, g-inner) gather -> yp[o, (t,h)] fp16
      (8-elem-run writes, cross-bank strided reads run at full ACT rate)
  T2 (DVE fp16 32x32 transpose): yp -> ypt[(d3,h), (t,o_hi)]
  S3: lhsT = I4 (x) Hpm again, rhs = ypt contiguous
      -> psum3[(d3',h'), (t,o_hi)]
  drain3 (ACT/DVE alternating): psum3 -> y_sb fp16 -> DMA out

Host: x cast to fp16 + column shuffle; output unshuffle + fp32 cast.
Sharding: data-parallel over tokens, 8 cores x 1024 tokens. No collectives.
"""

import sys

for _p in ("/opt/trn_rl_repo", "/root/.axon_site/_ro/trn_rl_repo"):
    if _p not in sys.path:
        sys.path.append(_p)

import numpy as np

import concourse.bass as bass  # noqa: E402
import concourse.mybir as mybir  # noqa: E402
import concourse.tile as tile  # noqa: E402
from concourse import bacc  # noqa: E402
from concourse.bass_utils import run_bass_kernel_spmd  # noqa: E402

F32 = mybir.dt.float32
F16 = mybir.dt.float16

N_CORES = 8
B, T, D = 4, 2048, 4096
A = 32              # algebra dim (hadamard size)
IN_O = 128
OUT_O = 128
TOK = (B * T) // N_CORES    # tokens per core = 1024
TC = 256                    # tokens per chunk
NCH = TOK // TC             # 4 chunks
CCOLS = TC * 32             # 8192 sbuf cols per chunk

_CACHE = {}


def _build_program():
    nc = bacc.Bacc(None, target_bir_lowering=False)

    x_d = nc.dram_tensor("x_shuf", [128, TOK * 32], F16, kind="ExternalInput")
    h1_d = nc.dram_tensor("h1m", [128, 128], F16, kind="ExternalInput")
    w2_d = nc.dram_tensor("w2m", [128, A * OUT_O], F16, kind="ExternalInput")
    y_d = nc.dram_tensor("y", [128, TOK * 32], F16, kind="ExternalOutput")

    with tile.TileContext(nc) as tc:
        with (
            tc.tile_pool(name="const", bufs=1) as constp,
            tc.tile_pool(name="xin", bufs=2) as xinp,
            tc.tile_pool(name="xm1", bufs=2) as xm1p,
            tc.tile_pool(name="xm2", bufs=1) as xm2p,
            tc.tile_pool(name="yp", bufs=2) as ypp,
            tc.tile_pool(name="ypt", bufs=1) as yptp,
            tc.tile_pool(name="ysb", bufs=1) as ysbp,
            tc.tile_pool(name="psA", bufs=2, space="PSUM") as psAp,
            tc.tile_pool(name="ps2", bufs=1, space="PSUM") as ps2p,
        ):
            h1_t = constp.tile([128, 128], F16)
            nc.sync.dma_start(out=h1_t[:], in_=h1_d[:])
            w2_t = constp.tile([128, A * OUT_O], F16)
            nc.sync.dma_start(out=w2_t[:], in_=w2_d[:])

            for c in range(NCH):
                # ---- load x chunk
                x_t = xinp.tile([128, CCOLS], F16)
                nc.sync.dma_start(
                    out=x_t[:], in_=x_d[:, c * CCOLS : (c + 1) * CCOLS]
                )

                # ---- S1 + drain1: 8 rounds of (2 MMs N=512 -> ACT drain)
                xm1_t = xm1p.tile([128, CCOLS], F16)
                for r in range(8):
                    ps = psAp.tile([128, 1024], F32, name="psq")
                    for m in range(2):
                        off = r * 1024 + m * 512
                        nc.tensor.matmul(
                            ps[:, m * 512 : m * 512 + 512],
                            h1_t[:],
                            x_t[:, off : off + 512],
                            start=True,
                            stop=True,
                        )
                    nc.scalar.copy(
                        xm1_t[:, r * 1024 : (r + 1) * 1024], ps[:]
                    )

                # ---- T1: paired fp32-view transpose (4 insts of 1024 f32cols)
                xm2_t = xm2p.tile([128, CCOLS], F16)
                xm1_32 = xm1_t[:].bitcast(F32)
                xm2_32 = xm2_t[:].bitcast(F32)
                for s in range(4):
                    nc.vector.transpose(
                        xm2_32[:, s * 1024 : (s + 1) * 1024],
                        xm1_32[:, s * 1024 : (s + 1) * 1024],
                    )

                # ---- S2 + gather2: 4 rounds of 8 g (N=256 each, 4 banks)
                yp_t = ypp.tile([128, CCOLS], F16)
                xm2_v = xm2_t[:].rearrange("p (u r) -> p u r", r=64)
                ypv = yp_t[:].rearrange("p (t h) -> p t h", h=32)
                for r in range(4):
                    ps2 = ps2p.tile([128, 2048], F32, name="ps2t")
                    for j in range(8):
                        g = r * 8 + j
                        nc.tensor.matmul(
                            ps2[:, j * 256 : (j + 1) * 256],
                            w2_t[:, g * 128 : (g + 1) * 128],
                            xm2_v[:, :, 2 * g : 2 * g + 2],
                            start=True,
                            stop=True,
                        )
                    # gather: yp[o, 32t + 8r + j] = ps2[o, 256j + t]
                    nc.scalar.copy(
                        ypv[:, :, 8 * r : 8 * r + 8],
                        ps2[:].rearrange("p (j t) -> p t j", j=8),
                    )

                # ---- T2: fp16 block transpose -> ypt[(d3,h), (t,o_hi)]
                ypt_t = yptp.tile([128, CCOLS], F16)
                for s in range(4):
                    nc.vector.transpose(
                        ypt_t[:, s * 2048 : (s + 1) * 2048],
                        yp_t[:, s * 2048 : (s + 1) * 2048],
                    )

                # ---- S3 + drain3: 8 rounds of (2 MMs N=512 -> drain)
                y_sb = ysbp.tile([128, CCOLS], F16)
                for r in range(8):
                    ps = psAp.tile([128, 1024], F32, name="psq")
                    for m in range(2):
                        off = r * 1024 + m * 512
                        nc.tensor.matmul(
                            ps[:, m * 512 : m * 512 + 512],
                            h1_t[:],
                            ypt_t[:, off : off + 512],
                            start=True,
                            stop=True,
                        )
                    dst = y_sb[:, r * 1024 : (r + 1) * 1024]
                    if r % 2 == 0:
                        nc.vector.tensor_copy(dst, ps[:])
                    else:
                        nc.scalar.copy(dst, ps[:])

                nc.sync.dma_start(
                    out=y_d[:, c * CCOLS : (c + 1) * CCOLS], in_=y_sb[:]
                )

    nc.compile()
    return nc


def _host_prep(x, weight_packed, beta, H):
    x = np.asarray(x, dtype=np.float32)
    weight_packed = np.asarray(weight_packed, dtype=np.uint8)
    beta = np.asarray(beta, dtype=np.float32)
    H = np.asarray(H, dtype=np.float32)

    hpm = np.where(H > 0, 1.0, -1.0).astype(np.float32)

    # unpack ternary weights exactly like the reference
    p = weight_packed
    v0 = ((p >> 6) & 3).astype(np.int8) - 1
    v1 = ((p >> 4) & 3).astype(np.int8) - 1
    v2 = ((p >> 2) & 3).astype(np.int8) - 1
    v3 = (p & 3).astype(np.int8) - 1
    w = np.stack([v0, v1, v2, v3], axis=-1).reshape(A, OUT_O, IN_O).astype(np.float32)

    # stage 1/3 stationary: h1m[32d+g, 32d'+h] = delta_dd' * hpm[g,h]
    h1m = np.zeros((4, A, 4, A), dtype=np.float32)
    for d in range(4):
        h1m[d, :, d, :] = hpm
    h1m = h1m.reshape(128, 128).astype(np.float16)

    # stage 2 stationary: w2m[i, 128g + o] = w[g,o,i] * beta[o] / 32
    w2 = w * (beta[None, :, None] / 32.0)
    w2m = np.ascontiguousarray(w2.transpose(2, 0, 1).reshape(IN_O, A * OUT_O)).astype(
        np.float16
    )

    # per-core pre-shuffled x (fp16):
    # x_shuf[32d+g, 8192c + 64u + 2*i_hi + v] = x[t0 + 256c + 2u + v, 128g+32d+i_hi]
    xf = x.reshape(B * T, D).astype(np.float16)
    x_shards = []
    for core in range(N_CORES):
        xc = xf[core * TOK : (core + 1) * TOK]          # [TOK, 4096]
        xc = xc.reshape(NCH, TC // 2, 2, A, 4, 32)      # [c, u, v, g, d, i_hi]
        # -> [d, g, c, u, i_hi, v]
        xc = xc.transpose(4, 3, 0, 1, 5, 2)
        x_shards.append(np.ascontiguousarray(xc.reshape(128, TOK * 32)))

    return x_shards, h1m, w2m


def kernel(x, weight_packed, beta, H):
    x_shards, h1m, w2m = _host_prep(x, weight_packed, beta, H)

    if "nc" not in _CACHE:
        _CACHE["nc"] = _build_program()
    nc = _CACHE["nc"]

    in_maps = [
        {"x_shuf": x_shards[c], "h1m": h1m, "w2m": w2m} for c in range(N_CORES)
    ]
    res = run_bass_kernel_spmd(nc, in_maps, core_ids=list(range(N_CORES)))

    # y_d[32d3'+h', 8192c + 32t + o_hi] = y[t0+256c+t, 128h' + 32d3' + o_hi]
    out = np.empty((B * T, D), dtype=np.float32)
    for core in range(N_CORES):
        yd = np.asarray(res.results[core]["y"])          # [128, TOK*32] fp16
        arr = yd.reshape(4, A, NCH, TC, 32)              # [d3', h', c, t, o_hi]
        arr = arr.transpose(2, 3, 1, 0, 4).reshape(TOK, D)  # [c,t][h',d3',o_hi]
        out[core * TOK : (core + 1) * TOK] = arr.astype(np.float32)
    return out.reshape(B, T, D)


# revision 5
# speedup vs baseline: 1.8196x; 1.3679x over previous
"""Trainium2 Bass kernel for HadamardPackedLinear.

Math (reference):
    y[t, 128*h + o] = beta[o]/32 * sum_g Hpm[g,h] * sum_i xm[t,g,i] * w[g,o,i]
    xm[t,g,i] = sum_g' x[t, 128g'+i] * Hpm[g',g]
with Hpm the +-1 Sylvester Hadamard (H = Hpm/sqrt(32), applied twice -> 1/32).

Three PE stages per 256-token chunk (4 chunks/core), everything fp16 on the
PE with fp32 PSUM accumulate:

  S1: lhsT = I4 (x) Hpm  [ (d,g) x (d,h) ],  rhs = x [ (d,g), (u,i_hi,v) ]
      (host pre-shuffles x; tokens t = 2u+v so fp16 pairs over v are
      memory-adjacent)  ->  psum1[(d,h), (u,i_hi,v)]
  drain1 (ACT): psum1 -> xm1 fp16, contiguous multibank copies
  T1 (DVE, fp32-bitcast pair transpose): xm1 -> xm2[i, (u,h,v)]
      (32x32 block transpose in fp32 units moves 2 fp16/lane-cycle)
  S2: per g: lhsT = w2'[g] = w[g].T * beta/32  [i, o],
      rhs = xm2 cols {64u + 2g + v}  (2-elem runs, stride 64)
      -> psum2[o, t] per g, 8 g per 4-bank psum tile
  gather2 (ACT): multibank (t-outer, g-inner) gather -> yp[o, (t,h)] fp16
      (8-elem-run writes, cross-bank strided reads run at full ACT rate)
  T2 (DVE fp16 32x32 transpose): yp -> ypt[(d3,h), (t,o_hi)]
  S3: lhsT = I4 (x) Hpm again, rhs = ypt contiguous
      -> psum3[(d3',h'), (t,o_hi)]
  drain3 (ACT/DVE alternating): psum3 -> y_sb fp16 -> DMA out

Host: x cast to fp16 + column shuffle; output unshuffle + fp32 cast.
Sharding: data-parallel over tokens, 8 cores x 1024 tokens. No collectives.
"""

import sys

for _p in ("/opt/trn_rl_repo", "/root/.axon_site/_ro/trn_rl_repo"):
    if _p not in sys.path:
        sys.path.append(_p)

import numpy as np

import concourse.bass as bass  # noqa: E402
import concourse.mybir as mybir  # noqa: E402
import concourse.tile as tile  # noqa: E402
from concourse import bacc  # noqa: E402
from concourse.bass_utils import run_bass_kernel_spmd  # noqa: E402

F32 = mybir.dt.float32
F16 = mybir.dt.float16

N_CORES = 8
B, T, D = 4, 2048, 4096
A = 32              # algebra dim (hadamard size)
IN_O = 128
OUT_O = 128
TOK = (B * T) // N_CORES    # tokens per core = 1024
TC = 256                    # tokens per chunk
NCH = TOK // TC             # 4 chunks
CCOLS = TC * 32             # 8192 sbuf cols per chunk

_CACHE = {}


def _build_program():
    nc = bacc.Bacc(None, target_bir_lowering=False)

    x_d = nc.dram_tensor("x_shuf", [128, TOK * 32], F16, kind="ExternalInput")
    h1_d = nc.dram_tensor("h1m", [128, 128], F16, kind="ExternalInput")
    w2_d = nc.dram_tensor("w2m", [128, A * OUT_O], F16, kind="ExternalInput")
    y_d = nc.dram_tensor("y", [128, TOK * 32], F16, kind="ExternalOutput")

    with tile.TileContext(nc) as tc:
        with (
            tc.tile_pool(name="const", bufs=1) as constp,
            tc.tile_pool(name="xin", bufs=2) as xinp,
            tc.tile_pool(name="xm1", bufs=1) as xm1p,
            tc.tile_pool(name="xm2", bufs=2) as xm2p,
            tc.tile_pool(name="yp", bufs=2) as ypp,
            tc.tile_pool(name="ypt", bufs=2) as yptp,
            tc.tile_pool(name="ysb", bufs=2) as ysbp,
            tc.tile_pool(name="psA", bufs=2, space="PSUM") as psAp,
            tc.tile_pool(name="ps2", bufs=1, space="PSUM") as ps2p,
        ):
            h1_t = constp.tile([128, 128], F16)
            nc.sync.dma_start(out=h1_t[:], in_=h1_d[:])
            w2_t = constp.tile([128, A * OUT_O], F16)
            nc.sync.dma_start(out=w2_t[:], in_=w2_d[:])

            def load_x(c):
                x_t = xinp.tile([128, CCOLS], F16, name="x_t")
                nc.sync.dma_start(
                    out=x_t[:], in_=x_d[:, c * CCOLS : (c + 1) * CCOLS]
                )
                return x_t

            def stage1(x_t):
                """S1 MMs + ACT drain + paired T1 -> returns xm2 tile."""
                xm1_t = xm1p.tile([128, CCOLS], F16, name="xm1_t")
                xm2_t = xm2p.tile([128, CCOLS], F16, name="xm2_t")
                xm1_32 = xm1_t[:].bitcast(F32)
                xm2_32 = xm2_t[:].bitcast(F32)
                for r in range(8):
                    ps = psAp.tile([128, 1024], F32, name="psq")
                    for m in range(2):
                        off = r * 1024 + m * 512
                        nc.tensor.matmul(
                            ps[:, m * 512 : m * 512 + 512],
                            h1_t[:],
                            x_t[:, off : off + 512],
                            start=True,
                            stop=True,
                        )
                    nc.scalar.copy(xm1_t[:, r * 1024 : (r + 1) * 1024], ps[:])
                    if r % 2 == 1:
                        s = r // 2
                        nc.vector.transpose(
                            xm2_32[:, s * 1024 : (s + 1) * 1024],
                            xm1_32[:, s * 1024 : (s + 1) * 1024],
                        )
                return xm2_t

            def stage2(xm2_t):
                """S2 MMs + ACT gather + T2 -> returns ypt tile."""
                yp_t = ypp.tile([128, CCOLS], F16, name="yp_t")
                ypt_t = yptp.tile([128, CCOLS], F16, name="ypt_t")
                xm2_v = xm2_t[:].rearrange("p (u r) -> p u r", r=64)
                ypv = yp_t[:].rearrange("p (t h) -> p t h", h=32)
                for r in range(4):
                    ps2 = ps2p.tile([128, 2048], F32, name="ps2t")
                    for j in range(8):
                        g = r * 8 + j
                        nc.tensor.matmul(
                            ps2[:, j * 256 : (j + 1) * 256],
                            w2_t[:, g * 128 : (g + 1) * 128],
                            xm2_v[:, :, 2 * g : 2 * g + 2],
                            start=True,
                            stop=True,
                        )
                    # gather: yp[o, 32t + 8r + j] = ps2[o, 256j + t]
                    nc.scalar.copy(
                        ypv[:, :, 8 * r : 8 * r + 8],
                        ps2[:].rearrange("p (j t) -> p t j", j=8),
                    )
                for s in range(4):
                    nc.vector.transpose(
                        ypt_t[:, s * 2048 : (s + 1) * 2048],
                        yp_t[:, s * 2048 : (s + 1) * 2048],
                    )
                return ypt_t

            def stage3(c, ypt_t):
                """S3 MMs + split drain + store."""
                y_sb = ysbp.tile([128, CCOLS], F16, name="y_sb")
                for r in range(8):
                    ps = psAp.tile([128, 1024], F32, name="psq")
                    for m in range(2):
                        off = r * 1024 + m * 512
                        nc.tensor.matmul(
                            ps[:, m * 512 : m * 512 + 512],
                            h1_t[:],
                            ypt_t[:, off : off + 512],
                            start=True,
                            stop=True,
                        )
                    dst = y_sb[:, r * 1024 : (r + 1) * 1024]
                    if r % 2 == 0:
                        nc.vector.tensor_copy(dst, ps[:])
                    else:
                        nc.scalar.copy(dst, ps[:])
                nc.sync.dma_start(
                    out=y_d[:, c * CCOLS : (c + 1) * CCOLS], in_=y_sb[:]
                )

            # software-pipelined schedule: S1(c) | S2(c-1) | S3(c-2)
            xs = {0: load_x(0)}
            xm2s = {}
            ypts = {}
            for it in range(NCH + 2):
                if it + 1 < NCH:
                    xs[it + 1] = load_x(it + 1)
                if it < NCH:
                    xm2s[it] = stage1(xs.pop(it))
                if 0 <= it - 1 < NCH:
                    ypts[it - 1] = stage2(xm2s.pop(it - 1))
                if 0 <= it - 2 < NCH:
                    stage3(it - 2, ypts.pop(it - 2))

    nc.compile()
    return nc


def _host_prep(x, weight_packed, beta, H):
    x = np.asarray(x, dtype=np.float32)
    weight_packed = np.asarray(weight_packed, dtype=np.uint8)
    beta = np.asarray(beta, dtype=np.float32)
    H = np.asarray(H, dtype=np.float32)

    hpm = np.where(H > 0, 1.0, -1.0).astype(np.float32)

    # unpack ternary weights exactly like the reference
    p = weight_packed
    v0 = ((p >> 6) & 3).astype(np.int8) - 1
    v1 = ((p >> 4) & 3).astype(np.int8) - 1
    v2 = ((p >> 2) & 3).astype(np.int8) - 1
    v3 = (p & 3).astype(np.int8) - 1
    w = np.stack([v0, v1, v2, v3], axis=-1).reshape(A, OUT_O, IN_O).astype(np.float32)

    # stage 1/3 stationary: h1m[32d+g, 32d'+h] = delta_dd' * hpm[g,h]
    h1m = np.zeros((4, A, 4, A), dtype=np.float32)
    for d in range(4):
        h1m[d, :, d, :] = hpm
    h1m = h1m.reshape(128, 128).astype(np.float16)

    # stage 2 stationary: w2m[i, 128g + o] = w[g,o,i] * beta[o] / 32
    w2 = w * (beta[None, :, None] / 32.0)
    w2m = np.ascontiguousarray(w2.transpose(2, 0, 1).reshape(IN_O, A * OUT_O)).astype(
        np.float16
    )

    # per-core pre-shuffled x (fp16):
    # x_shuf[32d+g, 8192c + 64u + 2*i_hi + v] = x[t0 + 256c + 2u + v, 128g+32d+i_hi]
    xf = x.reshape(B * T, D).astype(np.float16)
    x_shards = []
    for core in range(N_CORES):
        xc = xf[core * TOK : (core + 1) * TOK]          # [TOK, 4096]
        xc = xc.reshape(NCH, TC // 2, 2, A, 4, 32)      # [c, u, v, g, d, i_hi]
        # -> [d, g, c, u, i_hi, v]
        xc = xc.transpose(4, 3, 0, 1, 5, 2)
        x_shards.append(np.ascontiguousarray(xc.reshape(128, TOK * 32)))

    return x_shards, h1m, w2m


def kernel(x, weight_packed, beta, H):
    x_shards, h1m, w2m = _host_prep(x, weight_packed, beta, H)

    if "nc" not in _CACHE:
        _CACHE["nc"] = _build_program()
    nc = _CACHE["nc"]

    in_maps = [
        {"x_shuf": x_shards[c], "h1m": h1m, "w2m": w2m} for c in range(N_CORES)
    ]
    res = run_bass_kernel_spmd(nc, in_maps, core_ids=list(range(N_CORES)))

    # y_d[32d3'+h', 8192c + 32t + o_hi] = y[t0+256c+t, 128h' + 32d3' + o_hi]
    out = np.empty((B * T, D), dtype=np.float32)
    for core in range(N_CORES):
        yd = np.asarray(res.results[core]["y"])          # [128, TOK*32] fp16
        arr = yd.reshape(4, A, NCH, TC, 32)              # [d3', h', c, t, o_hi]
        arr = arr.transpose(2, 3, 1, 0, 4).reshape(TOK, D)  # [c,t][h',d3',o_hi]
        out[core * TOK : (core + 1) * TOK] = arr.astype(np.float32)
    return out.reshape(B, T, D)


# revision 6
# speedup vs baseline: 1.9912x; 1.0943x over previous
"""Trainium2 Bass kernel for HadamardPackedLinear.

Math (reference):
    y[t, 128*h + o] = beta[o]/32 * sum_g Hpm[g,h] * sum_i xm[t,g,i] * w[g,o,i]
    xm[t,g,i] = sum_g' x[t, 128g'+i] * Hpm[g',g]
with Hpm the +-1 Sylvester Hadamard (H = Hpm/sqrt(32), applied twice -> 1/32).

Three PE stages per 256-token chunk (4 chunks/core), everything fp16 on the
PE with fp32 PSUM accumulate:

  S1: lhsT = I4 (x) Hpm  [ (d,g) x (d,h) ],  rhs = x [ (d,g), (u,i_hi,v) ]
      (host pre-shuffles x; tokens t = 2u+v so fp16 pairs over v are
      memory-adjacent)  ->  psum1[(d,h), (u,i_hi,v)]
  drain1 (ACT): psum1 -> xm1 fp16, contiguous multibank copies
  T1 (DVE, fp32-bitcast pair transpose): xm1 -> xm2[i, (u,h,v)]
      (32x32 block transpose in fp32 units moves 2 fp16/lane-cycle)
  S2: per g: lhsT = w2'[g] = w[g].T * beta/32  [i, o],
      rhs = xm2 cols {64u + 2g + v}  (2-elem runs, stride 64)
      -> psum2[o, t] per g, 8 g per 4-bank psum tile
  gather2 (ACT): multibank (t-outer, g-inner) gather -> yp[o, (t,h)] fp16
      (8-elem-run writes, cross-bank strided reads run at full ACT rate)
  T2 (DVE fp16 32x32 transpose): yp -> ypt[(d3,h), (t,o_hi)]
  S3: lhsT = I4 (x) Hpm again, rhs = ypt contiguous
      -> psum3[(d3',h'), (t,o_hi)]
  drain3 (ACT/DVE alternating): psum3 -> y_sb fp16 -> DMA out

Host: x cast to fp16 + column shuffle; output unshuffle + fp32 cast.
Sharding: data-parallel over tokens, 8 cores x 1024 tokens. No collectives.
"""

import sys

for _p in ("/opt/trn_rl_repo", "/root/.axon_site/_ro/trn_rl_repo"):
    if _p not in sys.path:
        sys.path.append(_p)

import numpy as np

import concourse.bass as bass  # noqa: E402
import concourse.mybir as mybir  # noqa: E402
import concourse.tile as tile  # noqa: E402
from concourse import bacc  # noqa: E402
from concourse.bass_utils import run_bass_kernel_spmd  # noqa: E402

F32 = mybir.dt.float32
F16 = mybir.dt.float16

N_CORES = 8
B, T, D = 4, 2048, 4096
A = 32              # algebra dim (hadamard size)
IN_O = 128
OUT_O = 128
TOK = (B * T) // N_CORES    # tokens per core = 1024
TC = 256                    # tokens per chunk
NCH = TOK // TC             # 4 chunks
CCOLS = TC * 32             # 8192 sbuf cols per chunk

_CACHE = {}


def _build_program():
    nc = bacc.Bacc(None, target_bir_lowering=False)

    x_d = nc.dram_tensor("x_shuf", [128, TOK * 32], F16, kind="ExternalInput")
    h1_d = nc.dram_tensor("h1m", [128, 128], F16, kind="ExternalInput")
    w2_d = nc.dram_tensor("w2m", [128, A * OUT_O], F16, kind="ExternalInput")
    y_d = nc.dram_tensor("y", [128, TOK * 32], F16, kind="ExternalOutput")

    with tile.TileContext(nc) as tc:
        with (
            tc.tile_pool(name="const", bufs=1) as constp,
            tc.tile_pool(name="xin", bufs=2) as xinp,
            tc.tile_pool(name="xm1", bufs=1) as xm1p,
            tc.tile_pool(name="xm2", bufs=2) as xm2p,
            tc.tile_pool(name="yp", bufs=2) as ypp,
            tc.tile_pool(name="ypt", bufs=2) as yptp,
            tc.tile_pool(name="ysb", bufs=2) as ysbp,
            tc.tile_pool(name="psA", bufs=2, space="PSUM") as psAp,
            tc.tile_pool(name="ps2", bufs=2, space="PSUM") as ps2p,
        ):
            h1_t = constp.tile([128, 128], F16)
            nc.sync.dma_start(out=h1_t[:], in_=h1_d[:])
            w2_t = constp.tile([128, A * OUT_O], F16)
            nc.sync.dma_start(out=w2_t[:], in_=w2_d[:])

            def load_x(c):
                x_t = xinp.tile([128, CCOLS], F16, name="x_t")
                nc.sync.dma_start(
                    out=x_t[:], in_=x_d[:, c * CCOLS : (c + 1) * CCOLS]
                )
                return x_t

            def stage1(x_t):
                """S1 MMs + ACT drain + paired T1 -> returns xm2 tile."""
                xm1_t = xm1p.tile([128, CCOLS], F16, name="xm1_t")
                xm2_t = xm2p.tile([128, CCOLS], F16, name="xm2_t")
                xm1_32 = xm1_t[:].bitcast(F32)
                xm2_32 = xm2_t[:].bitcast(F32)
                for r in range(8):
                    ps = psAp.tile([128, 1024], F32, name="psq")
                    for m in range(2):
                        off = r * 1024 + m * 512
                        nc.tensor.matmul(
                            ps[:, m * 512 : m * 512 + 512],
                            h1_t[:],
                            x_t[:, off : off + 512],
                            start=True,
                            stop=True,
                        )
                    nc.scalar.copy(xm1_t[:, r * 1024 : (r + 1) * 1024], ps[:])
                    if r % 2 == 1:
                        s = r // 2
                        nc.vector.transpose(
                            xm2_32[:, s * 1024 : (s + 1) * 1024],
                            xm1_32[:, s * 1024 : (s + 1) * 1024],
                        )
                return xm2_t

            def stage2(xm2_t):
                """S2 MMs + ACT gather + T2 -> returns ypt tile."""
                yp_t = ypp.tile([128, CCOLS], F16, name="yp_t")
                ypt_t = yptp.tile([128, CCOLS], F16, name="ypt_t")
                xm2_v = xm2_t[:].rearrange("p (u r) -> p u r", r=64)
                ypv = yp_t[:].rearrange("p (t h) -> p t h", h=32)
                for r in range(8):
                    ps2 = ps2p.tile([128, 1024], F32, name="ps2t")
                    for j in range(4):
                        g = r * 4 + j
                        nc.tensor.matmul(
                            ps2[:, j * 256 : (j + 1) * 256],
                            w2_t[:, g * 128 : (g + 1) * 128],
                            xm2_v[:, :, 2 * g : 2 * g + 2],
                            start=True,
                            stop=True,
                        )
                    # gather: yp[o, 32t + 4r + j] = ps2[o, 256j + t]
                    nc.scalar.copy(
                        ypv[:, :, 4 * r : 4 * r + 4],
                        ps2[:].rearrange("p (j t) -> p t j", j=4),
                    )
                for s in range(4):
                    nc.vector.transpose(
                        ypt_t[:, s * 2048 : (s + 1) * 2048],
                        yp_t[:, s * 2048 : (s + 1) * 2048],
                    )
                return ypt_t

            def stage3(c, ypt_t):
                """S3 MMs + split drain + store."""
                y_sb = ysbp.tile([128, CCOLS], F16, name="y_sb")
                for r in range(8):
                    ps = psAp.tile([128, 1024], F32, name="psq")
                    for m in range(2):
                        off = r * 1024 + m * 512
                        nc.tensor.matmul(
                            ps[:, m * 512 : m * 512 + 512],
                            h1_t[:],
                            ypt_t[:, off : off + 512],
                            start=True,
                            stop=True,
                        )
                    dst = y_sb[:, r * 1024 : (r + 1) * 1024]
                    if r in (1, 4, 7):
                        nc.scalar.copy(dst, ps[:])
                    else:
                        nc.vector.tensor_copy(dst, ps[:])
                nc.sync.dma_start(
                    out=y_d[:, c * CCOLS : (c + 1) * CCOLS], in_=y_sb[:]
                )

            # software-pipelined schedule: S1(c) | S2(c-1) | S3(c-2)
            xs = {0: load_x(0)}
            xm2s = {}
            ypts = {}
            for it in range(NCH + 2):
                if it + 1 < NCH:
                    xs[it + 1] = load_x(it + 1)
                if it < NCH:
                    xm2s[it] = stage1(xs.pop(it))
                if 0 <= it - 1 < NCH:
                    ypts[it - 1] = stage2(xm2s.pop(it - 1))
                if 0 <= it - 2 < NCH:
                    stage3(it - 2, ypts.pop(it - 2))

    nc.compile()
    return nc


def _host_prep(x, weight_packed, beta, H):
    x = np.asarray(x, dtype=np.float32)
    weight_packed = np.asarray(weight_packed, dtype=np.uint8)
    beta = np.asarray(beta, dtype=np.float32)
    H = np.asarray(H, dtype=np.float32)

    hpm = np.where(H > 0, 1.0, -1.0).astype(np.float32)

    # unpack ternary weights exactly like the reference
    p = weight_packed
    v0 = ((p >> 6) & 3).astype(np.int8) - 1
    v1 = ((p >> 4) & 3).astype(np.int8) - 1
    v2 = ((p >> 2) & 3).astype(np.int8) - 1
    v3 = (p & 3).astype(np.int8) - 1
    w = np.stack([v0, v1, v2, v3], axis=-1).reshape(A, OUT_O, IN_O).astype(np.float32)

    # stage 1/3 stationary: h1m[32d+g, 32d'+h] = delta_dd' * hpm[g,h]
    h1m = np.zeros((4, A, 4, A), dtype=np.float32)
    for d in range(4):
        h1m[d, :, d, :] = hpm
    h1m = h1m.reshape(128, 128).astype(np.float16)

    # stage 2 stationary: w2m[i, 128g + o] = w[g,o,i] * beta[o] / 32
    w2 = w * (beta[None, :, None] / 32.0)
    w2m = np.ascontiguousarray(w2.transpose(2, 0, 1).reshape(IN_O, A * OUT_O)).astype(
        np.float16
    )

    # per-core pre-shuffled x (fp16):
    # x_shuf[32d+g, 8192c + 64u + 2*i_hi + v] = x[t0 + 256c + 2u + v, 128g+32d+i_hi]
    xf = x.reshape(B * T, D).astype(np.float16)
    x_shards = []
    for core in range(N_CORES):
        xc = xf[core * TOK : (core + 1) * TOK]          # [TOK, 4096]
        xc = xc.reshape(NCH, TC // 2, 2, A, 4, 32)      # [c, u, v, g, d, i_hi]
        # -> [d, g, c, u, i_hi, v]
        xc = xc.transpose(4, 3, 0, 1, 5, 2)
        x_shards.append(np.ascontiguousarray(xc.reshape(128, TOK * 32)))

    return x_shards, h1m, w2m


def kernel(x, weight_packed, beta, H):
    x_shards, h1m, w2m = _host_prep(x, weight_packed, beta, H)

    if "nc" not in _CACHE:
        _CACHE["nc"] = _build_program()
    nc = _CACHE["nc"]

    in_maps = [
        {"x_shuf": x_shards[c], "h1m": h1m, "w2m": w2m} for c in range(N_CORES)
    ]
    res = run_bass_kernel_spmd(nc, in_maps, core_ids=list(range(N_CORES)))

    # y_d[32d3'+h', 8192c + 32t + o_hi] = y[t0+256c+t, 128h' + 32d3' + o_hi]
    out = np.empty((B * T, D), dtype=np.float32)
    for core in range(N_CORES):
        yd = np.asarray(res.results[core]["y"])          # [128, TOK*32] fp16
        arr = yd.reshape(4, A, NCH, TC, 32)              # [d3', h', c, t, o_hi]
        arr = arr.transpose(2, 3, 1, 0, 4).reshape(TOK, D)  # [c,t][h',d3',o_hi]
        out[core * TOK : (core + 1) * TOK] = arr.astype(np.float32)
    return out.reshape(B, T, D)
